# revision 7
# baseline (speedup 1.0000x reference)
"""GAT layer kernel for Trainium2 (8 NeuronCores, SPMD).

Math note: in the reference,
    att = softmax(scores, axis=1); w = att.sum(axis=1)
sums the softmax over the exact axis it normalizes, so w == 1 identically
(up to fp rounding).  The layer therefore reduces to
    out[v] = (1/H) * ( (sum_{e: dst[e]==v} x[src[e]]) @ W  +  deg_in(v) * b )
i.e. a sparse neighbor aggregation (gather + segment-sum) followed by a
small dense matmul.  This is memory-bound in the per-edge row gather.

Sharding: edges are partitioned by dst-node range (6250 nodes per core), so
each core owns the full accumulation for its node slice and the final
output is a pure concatenation -- no inter-core collective needed.

Device pipeline per core:
  - edges are grouped by (128-node output group, src half); gather indices
    are int16, so x is split in two row halves.  Each (group, half) chunk
    is one dma_gather; trailing pad slots carry idx -1, which the SWDGE
    ucode trims, so pads are NOT fetched (0% fetch padding).  Pad slots
    carry dst offset -1 so stale buffer data contributes exactly nothing
    to the MACs (buffers are memset once at start so stale data is finite).
  - the x table is fp16: halves HBM bytes; accumulation stays fp32.
  - gathers are spread over 4 SWDGE queues with greedy load balancing;
    queue q runs on Q7 DSP pair (2q, 2q+1), so the 4 queues' descriptor
    generation runs concurrently.  Deep tile-pool buffering (16 gather
    buffers) keeps all queue pairs busy; the remaining floor is the DMA
    engines' per-packet rate (~16.7 ns per 256B row across 16 engines).
  - segment-sum on the tensor engine, one matmul per 128-edge subtile:
      ypsum_g[f, v] += xg[e, f]^T @ onehot[e, v]
    with the one-hot built by a vector-engine is_equal of an int8 iota row
    against the per-edge group-local dst offset, output in fp8e4 (exact
    0.0/1.0), consumed by a mixed fp16 x fp8 matmul.
  - per group, PSUM y is copied to fp16 and multiplied by W/4 (fp16,
    K=128 matmul) plus a K=1 fp32 outer-product matmul for the deg*b/4
    term, and streamed out.
"""

import numpy as np

import concourse.bass as bass
import concourse.tile as tile
from concourse import bacc, mybir
from concourse.bass_utils import run_bass_kernel_spmd

F32 = mybir.dt.float32
F16 = mybir.dt.float16
I16 = mybir.dt.int16
I8 = mybir.dt.int8
OHDT = mybir.dt.float16   # one-hot dtype
GDT = mybir.dt.float16     # gather-table dtype
GNP = np.float16
ODNP = np.float16          # dstoff/iota host dtype
ODT = mybir.dt.float16     # dstoff/iota device dtype
WDT = mybir.dt.float16     # w4 dtype
WNP = np.float16
YDT = mybir.dt.float16     # ysb dtype

N_NODES = 50000
N_EDGES = 800000
D = 128          # in feats == H*F
HEADS = 4
N_CORES = 8
NPC = N_NODES // N_CORES      # nodes per core
P = 128                       # partitions / node-group size
GBUFS = 16                    # gather buffers in flight
NQ = 4                        # SWDGE queues (DSP pairs)


def _prep(x, weight, bias, src, dst, n_cores, npc):
    """Host-side sharding; returns per-core input maps + static tiling."""
    n_nodes, d = x.shape
    ng = (npc + P - 1) // P
    half = (n_nodes + 1) // 2
    assert half < 32768 and (n_nodes - half) < 32768

    src64 = src.astype(np.int64)
    dst64 = dst.astype(np.int64)
    core = dst64 // npc
    loc_node = dst64 % npc
    g_e = loc_node // P
    loc = (loc_node % P).astype(ODNP)
    h_e = (src64 >= half).astype(np.int64)

    key = (core * ng + g_e) * 2 + h_e
    order = np.argsort(key, kind="stable")
    key_s = key[order]
    src_s = src64[order]
    loc_s = loc[order]

    cnt = np.bincount(key, minlength=n_cores * ng * 2).reshape(n_cores, ng, 2)
    nk2 = (-(-cnt.max(axis=0) // P)).astype(np.int64)  # [ng, 2] tiles/chunk

    # chunk list in issue order with greedy queue balancing
    seg_t0 = np.zeros((ng, 2), np.int64)
    chunks = []  # (g, hh, t0, ntiles, queue)
    qload = [0] * NQ
    t = 0
    for g in range(ng):
        for hh in (0, 1):
            nt = int(nk2[g, hh])
            seg_t0[g, hh] = t
            if nt == 0:
                continue
            qn = min(range(NQ), key=lambda q: qload[q])
            qload[qn] += nt
            chunks.append((g, hh, t, nt, qn))
            t += nt
    T = t
    cap = int(nk2.max())

    seg_start = np.zeros(n_cores * ng * 2, np.int64)
    seg_start[1:] = np.cumsum(cnt.reshape(-1))[:-1]
    rank = np.arange(len(src_s), dtype=np.int64) - seg_start[key_s]
    c_e = key_s // (ng * 2)
    gg_e = (key_s // 2) % ng
    t_e = seg_t0[gg_e, key_s % 2] + rank // P
    p_e = rank % P

    dstoff = np.full((n_cores, P, T), -1, ODNP)
    dstoff[c_e, p_e, t_e] = loc_s

    # int16 indices: slot (p, t) -> idx16[p % 16, 8*t + p//16]; pads = -1
    # (trailing negatives are trimmed by the gather ucode -> not fetched)
    idx16 = np.full((n_cores, 16, 8 * T), -1, np.int16)
    hval = (src_s - (key_s % 2) * half).astype(np.int16)
    idx16[c_e, p_e % 16, 8 * t_e + p_e // 16] = hval
    idx16 = np.ascontiguousarray(np.tile(idx16, (1, 8, 1)))

    deg = np.bincount(dst64, minlength=n_nodes).astype(np.float32)
    deg4 = np.zeros((n_cores, 1, ng * P), np.float32)
    deg4[:, 0, :npc] = (deg / HEADS).reshape(n_cores, npc)

    # per-core, per-chunk tile-aligned index counts for num_idxs_reg.
    # The SWDGE decode reserves ring slots from the register value while the
    # Q7 ucode trims trailing -1 idxs and emits ceil(count/128)*128
    # descriptors -- the two must match exactly or the rings desync.
    nidx = np.zeros((n_cores, 1, len(chunks)), np.int32)
    for ci, (g, hh, _t0, _nt, _qn) in enumerate(chunks):
        nidx[:, 0, ci] = -(-cnt[:, g, hh] // P) * P

    iota = np.broadcast_to(np.arange(P, dtype=ODNP)[None, :], (P, P)).copy()
    w4 = np.ascontiguousarray((weight.astype(np.float32) / HEADS).astype(WNP))
    b4 = bias.astype(np.float32).reshape(1, d)  # deg4 already carries the /H
    xg16 = np.ascontiguousarray(x.astype(GNP))
    xlo = np.ascontiguousarray(xg16[:half])
    xhi = np.ascontiguousarray(xg16[half:])

    in_maps = []
    for c in range(n_cores):
        in_maps.append(
            {
                "xlo": xlo,
                "xhi": xhi,
                "idx": idx16[c],
                "dstoff": dstoff[c],
                "iota": iota,
                "w4": w4,
                "b4": b4,
                "deg4": deg4[c],
                "nidx": nidx[c],
            }
        )
    meta = dict(nk2=nk2, seg_t0=seg_t0, chunks=chunks, T=T, ng=ng,
                cap=cap, half=half)
    return in_maps, meta


def _build(n_nodes, d, npc, meta):
    nk2, seg_t0, chunks, T, ng, cap, half = (
        meta["nk2"], meta["seg_t0"], meta["chunks"], meta["T"],
        meta["ng"], meta["cap"], meta["half"],
    )
    nc = bacc.Bacc("TRN2", num_swdge_queues=NQ)
    xlo_d = nc.dram_tensor("xlo", [half, d], GDT, kind="ExternalInput")
    xhi_d = nc.dram_tensor("xhi", [n_nodes - half, d], GDT, kind="ExternalInput")
    idx_d = nc.dram_tensor("idx", [P, 8 * T], I16, kind="ExternalInput")
    dstoff_d = nc.dram_tensor("dstoff", [P, T], ODT, kind="ExternalInput")
    iota_d = nc.dram_tensor("iota", [P, P], ODT, kind="ExternalInput")
    w4_d = nc.dram_tensor("w4", [d, d], WDT, kind="ExternalInput")
    b4_d = nc.dram_tensor("b4", [1, d], F32, kind="ExternalInput")
    deg4_d = nc.dram_tensor("deg4", [1, ng * P], F32, kind="ExternalInput")
    nidx_d = nc.dram_tensor("nidx", [1, len(chunks)], mybir.dt.int32,
                            kind="ExternalInput")
    out_d = nc.dram_tensor("out", [npc, d], F32, kind="ExternalOutput")

    x_in = [xlo_d, xhi_d]
    chunk_of = {}
    for (g, hh, t0c, nt, qn) in chunks:
        chunk_of[(g, hh)] = (t0c, nt, qn)

    with tile.TileContext(nc) as tc:
        with (
            tc.tile_pool(name="consts", bufs=1) as cpool,
            tc.tile_pool(name="xg", bufs=GBUFS) as gpool,
            tc.tile_pool(name="ind", bufs=16) as ipool,
            tc.tile_pool(name="ysb", bufs=3) as ypool,
            tc.tile_pool(name="osb", bufs=3) as opool,
            tc.tile_pool(name="ypsum", bufs=6, space="PSUM") as yppool,
            tc.tile_pool(name="opsum", bufs=2, space="PSUM") as oppool,
        ):
            # piecewise index tables as INDEPENDENT tiles (Tile tracks deps
            # per tile, so the first gather must not wait on the whole
            # table).  Piece boundaries align with chunk boundaries.
            npiece = 10
            target = -(-T // npiece)
            bndl = [0]
            for (_g, _hh, t0c, nt, _qn) in chunks:
                if t0c + nt - bndl[-1] >= target and t0c + nt < T:
                    bndl.append(t0c + nt)
            bndl.append(T)
            idx_p, dst_p = [], []
            for i in range(len(bndl) - 1):
                lo, hi = bndl[i], bndl[i + 1]
                ip = cpool.tile([P, 8 * (hi - lo)], I16, name=f"idxp{i}")
                nc.sync.dma_start(out=ip[:], in_=idx_d[:, 8 * lo : 8 * hi])
                dp = cpool.tile([P, hi - lo], ODT, name=f"dstp{i}")
                nc.sync.dma_start(out=dp[:], in_=dstoff_d[:, lo:hi])
                idx_p.append(ip)
                dst_p.append(dp)

            def piece_of(t):
                for i in range(len(bndl) - 1):
                    if bndl[i] <= t < bndl[i + 1]:
                        return i, bndl[i]
                raise AssertionError(t)

            def _idxs_ap_of(t0c, nt):
                pi, pb = piece_of(t0c)
                assert t0c + nt <= bndl[pi + 1], "chunk straddles piece"
                return idx_p[pi][:, 8 * (t0c - pb) : 8 * (t0c - pb + nt)]

            iota_sb = cpool.tile([P, P], ODT)
            nc.sync.dma_start(out=iota_sb[:], in_=iota_d[:])
            w4_sb = cpool.tile([d, d], WDT)
            nc.sync.dma_start(out=w4_sb[:], in_=w4_d[:])
            b4_sb = cpool.tile([1, d], F32)
            nc.sync.dma_start(out=b4_sb[:], in_=b4_d[:])
            deg4_sb = cpool.tile([1, ng * P], F32)
            nc.sync.dma_start(out=deg4_sb[:], in_=deg4_d[:])
            nidx_sb = cpool.tile([1, len(chunks)], mybir.dt.int32)
            nc.sync.dma_start(out=nidx_sb[:], in_=nidx_d[:])
            # num_idxs registers: double-buffered blocks so the block
            # reg_load only WARs against gathers two blocks back (long
            # retired) instead of stalling the pipeline every chunk.
            NREGS = 12
            nidx_sets = [
                [nc.gpsimd.alloc_register(f"nidx_reg{s}_{i}")
                 for i in range(NREGS)]
                for s in (0, 1)
            ]
            NCH = len(chunks)
            ci_of = {}
            for ci, (g, hh, _t0, _nt, _qn) in enumerate(chunks):
                ci_of[(g, hh)] = ci

            # memset the gather buffers once so pad slots (never fetched)
            # hold finite data; 0.0 * onehot(-1)==0 keeps them inert.
            for i in range(GBUFS):
                zt = gpool.tile([P, cap * P], GDT, tag="xg", name=f"xgz{i}")
                nc.vector.memset(zt[:], 0.0)

            for g in range(ng):
                n_sub = int(nk2[g][0] + nk2[g][1])
                if n_sub == 0:
                    continue
                ypsum = yppool.tile([P, P], F32, tag="ypsum", name=f"ypsum_{g}")
                xgc = {}
                for hh in (0, 1):
                    if (g, hh) not in chunk_of:
                        continue
                    t0c, nt, qn = chunk_of[(g, hh)]
                    xgc[hh] = gpool.tile([P, cap * P], GDT, tag="xg",
                                         name=f"xg_{g}_{hh}")
                    ci = ci_of[(g, hh)]
                    if ci % NREGS == 0:
                        blk = ci // NREGS
                        n = min(NREGS, NCH - ci)
                        nc.gpsimd.reg_load(
                            nidx_sets[blk % 2][:n],
                            nidx_sb[0:1, ci : ci + n],
                        )
                    nidx_reg = nidx_sets[(ci // NREGS) % 2][ci % NREGS]
                    nc.gpsimd.dma_gather(
                        out_ap=xgc[hh][:, : nt * P].rearrange(
                            "p (k f) -> p k f", f=P
                        ),
                        in_ap=x_in[hh][:],
                        idxs_ap=_idxs_ap_of(t0c, nt),
                        num_idxs=nt * P,
                        num_idxs_reg=nidx_reg,
                        elem_size=P,
                        queue_num=qn,
                        single_packet=False,
                    )

                done = 0
                for hh in (0, 1):
                    if (g, hh) not in chunk_of:
                        continue
                    t0c, nt, _qn = chunk_of[(g, hh)]
                    for k in range(nt):
                        t = t0c + k
                        ind = ipool.tile([P, P], OHDT, tag="ind",
                                         name=f"ind_{t}")
                        pi, pb = piece_of(t)
                        nc.vector.tensor_tensor(
                            out=ind[:],
                            in0=dst_p[pi][:, t - pb : t - pb + 1]
                            .to_broadcast([P, P]),
                            in1=iota_sb[:],
                            op=mybir.AluOpType.is_equal,
                        )
                        nc.tensor.matmul(
                            out=ypsum[:],
                            lhsT=xgc[hh][:, k * P : (k + 1) * P],
                            rhs=ind[:],
                            start=(done == 0),
                            stop=(done == n_sub - 1),
                        )
                        done += 1

                nv = min(P, npc - g * P)
                ysb = ypool.tile([P, P], YDT, tag="ysb", name=f"ysb_{g}")
                nc.scalar.copy(out=ysb[:], in_=ypsum[:])
                opsum = oppool.tile([P, P], F32, tag="opsum",
                                    name=f"opsum_{g}")
                nc.tensor.matmul(
                    out=opsum[:], lhsT=ysb[:], rhs=w4_sb[:],
                    start=True, stop=False,
                )
                nc.tensor.matmul(
                    out=opsum[:],
                    lhsT=deg4_sb[:, g * P : (g + 1) * P],
                    rhs=b4_sb[:],
                    start=False,
                    stop=True,
                )
                osb = opool.tile([P, P], F32, tag="osb", name=f"osb_{g}")
                nc.scalar.copy(out=osb[:], in_=opsum[:])
                nc.sync.dma_start(
                    out=out_d[g * P : g * P + nv, :], in_=osb[:nv, :]
                )

    nc.compile()
    return nc


def _run(inputs, trace=False, trace_kwargs=None):
    x = np.asarray(inputs["x"], np.float32)
    weight = np.asarray(inputs["weight"], np.float32)
    bias = np.asarray(inputs["bias"], np.float32)
    src = np.asarray(inputs["src"])
    dst = np.asarray(inputs["dst"])

    in_maps, meta = _prep(x, weight, bias, src, dst, N_CORES, NPC)
    nc = _build(N_NODES, D, NPC, meta)
    res = run_bass_kernel_spmd(
        nc,
        in_maps,
        list(range(N_CORES)),
        trace=trace,
        **(trace_kwargs or {}),
    )
    out = np.concatenate([res.results[c]["out"] for c in range(N_CORES)], axis=0)
    return out.reshape(N_NODES, HEADS, D // HEADS), res


def kernel(**inputs) -> np.ndarray:
    # the device occasionally comes up wedged from a prior run
    # (NRT_EXEC_UNIT_UNRECOVERABLE); a retry recovers it
    last = None
    for _ in range(3):
        try:
            out, _ = _run(inputs)
            return out
        except Exception as e:  # noqa: BLE001
            last = e
    raise last


# revision 9
# speedup vs baseline: 1.1725x; 1.1725x over previous
"""GAT layer kernel for Trainium2 (8 NeuronCores, SPMD).

Math note: in the reference,
    att = softmax(scores, axis=1); w = att.sum(axis=1)
sums the softmax over the exact axis it normalizes, so w == 1 identically
(up to fp rounding).  The layer therefore reduces to
    out[v] = (1/H) * ( (sum_{e: dst[e]==v} x[src[e]]) @ W  +  deg_in(v) * b )
i.e. a sparse neighbor aggregation (gather + segment-sum) followed by a
small dense matmul.  This is memory-bound in the per-edge row gather.

Sharding: edges are partitioned by dst-node range (6250 nodes per core), so
each core owns the full accumulation for its node slice and the final
output is a pure concatenation -- no inter-core collective needed.

Device pipeline per core (bottleneck structure, measured):
  - the SWDGE gather's idx ingestion is serialized on the Pool engine's
    shared read stream (~1.1 ns/idx), so the gather phase floor is set by
    total ingested idx slots; the DMA engines add ~20 ns per 256B row
    (16 engines).  Each dma_gather also burns a DMA-completion semaphore
    from a small rotating pool whose in-order reset waits on full DMA
    completion of the reusing gather's predecessor -- fewer, bigger
    gathers pipeline much better than many small ones.
  - edges are grouped by (2-group bundle, src half): 50 chunks, one
    dma_gather each, spread over 4 SWDGE queues (queue q = Q7 DSP pair
    2q,2q+1) with greedy load balancing.  Within a chunk the two groups'
    segments are 128-aligned; interior pad slots gather row 0, trailing
    pad slots carry idx -1 which the ucode trims (not fetched).  The
    per-core trimmed descriptor count is passed via num_idxs_reg
    (register, loaded in blocks of 12) so the decode-side ring
    bookkeeping matches the Q7-side generation exactly.
  - the x table is fp16: halves HBM bytes; accumulation stays fp32.
  - segment-sum on the tensor engine, one matmul per 128-edge subtile:
      ypsum_g[f, v] += xg[e, f]^T @ onehot[e, v]
    with ALL of a chunk's subtile one-hots built by a single wide
    vector-engine is_equal (dstoff column-block broadcast against an
    iota row), amortizing DVE instruction overhead ~10x.
  - per group, PSUM y is copied to fp16 and multiplied by W/4 (fp16,
    K=128 matmul) plus a K=1 fp16 outer-product matmul for the deg*b/4
    term (fp32 K=1 matmuls cost ~1.6us of PE each; fp16 is exact enough:
    deg/4 is a small exact quarter-integer), and streamed out.
"""

import numpy as np

import concourse.bass as bass
import concourse.tile as tile
from concourse import bacc, mybir
from concourse.bass_utils import run_bass_kernel_spmd

F32 = mybir.dt.float32
F16 = mybir.dt.float16
I16 = mybir.dt.int16
I32 = mybir.dt.int32
OHDT = mybir.dt.float16    # one-hot dtype
GDT = mybir.dt.float16     # gather-table dtype
GNP = np.float16
ODNP = np.float16          # dstoff/iota host dtype
ODT = mybir.dt.float16     # dstoff/iota device dtype

N_NODES = 50000
N_EDGES = 800000
D = 128          # in feats == H*F
HEADS = 4
N_CORES = 8
NPC = N_NODES // N_CORES      # nodes per core
P = 128                       # partitions / node-group size
BUNDLE = 2                    # groups per gather chunk
GBUFS = 12                    # gather buffers in flight
NQ = 4                        # SWDGE queues (DSP pairs)
NREGS = 12                    # num_idxs register block size


def _prep(x, weight, bias, src, dst, n_cores, npc):
    """Host-side sharding; returns per-core input maps + static tiling."""
    n_nodes, d = x.shape
    ng = (npc + P - 1) // P
    nb = (ng + BUNDLE - 1) // BUNDLE
    half = (n_nodes + 1) // 2
    assert half < 32768 and (n_nodes - half) < 32768

    src64 = src.astype(np.int64)
    dst64 = dst.astype(np.int64)
    core = dst64 // npc
    loc_node = dst64 % npc
    g_e = loc_node // P
    loc = (loc_node % P).astype(ODNP)
    h_e = (src64 >= half).astype(np.int64)

    key = (core * ng + g_e) * 2 + h_e
    order = np.argsort(key, kind="stable")
    key_s = key[order]
    src_s = src64[order]
    loc_s = loc[order]

    cnt = np.bincount(key, minlength=n_cores * ng * 2).reshape(n_cores, ng, 2)
    nk2 = (-(-cnt.max(axis=0) // P)).astype(np.int64)  # [ng, 2] tiles/segment

    # chunk list: (bundle, half) in issue order with greedy queue balance.
    # Within a chunk the member groups' segments are laid out consecutively
    # (128-aligned each).
    seg_t0 = np.zeros((ng, 2), np.int64)
    chunks = []  # (b, hh, t0, ntiles, queue, groups)
    qload = [0] * NQ
    t = 0
    for b in range(nb):
        groups = list(range(b * BUNDLE, min(ng, (b + 1) * BUNDLE)))
        for hh in (0, 1):
            t0c = t
            for g in groups:
                seg_t0[g, hh] = t
                t += int(nk2[g, hh])
            nt = t - t0c
            if nt == 0:
                continue
            qn = min(range(NQ), key=lambda q: qload[q])
            qload[qn] += nt
            chunks.append((b, hh, t0c, nt, qn, groups))
    T = t
    cap = max(nt for (_b, _hh, _t0, nt, _qn, _gr) in chunks)

    seg_start = np.zeros(n_cores * ng * 2, np.int64)
    seg_start[1:] = np.cumsum(cnt.reshape(-1))[:-1]
    rank = np.arange(len(src_s), dtype=np.int64) - seg_start[key_s]
    c_e = key_s // (ng * 2)
    gg_e = (key_s // 2) % ng
    t_e = seg_t0[gg_e, key_s % 2] + rank // P
    p_e = rank % P

    dstoff = np.full((n_cores, P, T), -1, ODNP)
    dstoff[c_e, p_e, t_e] = loc_s

    # int16 indices: slot (p, t) -> idx16[p % 16, 8*t + p//16].
    # Interior pads (non-final segment of a chunk) fetch row 0 (idx 0,
    # dstoff -1 keeps them inert); trailing pads of the final segment are
    # -1 (trimmed by the ucode -> not fetched).  The per-core trimmed
    # count goes to nidx for num_idxs_reg.
    idx16 = np.zeros((n_cores, 16, 8 * T), np.int16)
    hval = (src_s - (key_s % 2) * half).astype(np.int16)

    nidx = np.zeros((n_cores, 1, len(chunks)), np.int32)
    for ci, (b, hh, t0c, nt, _qn, groups) in enumerate(chunks):
        gl = groups[-1]
        last_t0 = int(seg_t0[gl, hh])          # global tile of last seg
        seg_tiles = int(nk2[gl, hh])
        for c in range(n_cores):
            cl = int(cnt[c, gl, hh])
            nidx[c, 0, ci] = (last_t0 - t0c) * P + cl
            for pos in range(cl, seg_tiles * P):
                tt, pp = last_t0 + pos // P, pos % P
                idx16[c, pp % 16, 8 * tt + pp // 16] = -1

    idx16[c_e, p_e % 16, 8 * t_e + p_e // 16] = hval
    idx16 = np.ascontiguousarray(np.tile(idx16, (1, 8, 1)))

    deg = np.bincount(dst64, minlength=n_nodes).astype(np.float32)
    deg4 = np.zeros((n_cores, 1, ng * P), GNP)
    deg4[:, 0, :npc] = (deg / HEADS).reshape(n_cores, npc).astype(GNP)

    iota = np.broadcast_to(np.arange(P, dtype=ODNP)[None, :], (P, P)).copy()
    w4 = np.ascontiguousarray((weight.astype(np.float32) / HEADS).astype(GNP))
    b4 = bias.astype(np.float32).reshape(1, d).astype(GNP)
    xg16 = np.ascontiguousarray(x.astype(GNP))
    xlo = np.ascontiguousarray(xg16[:half])
    xhi = np.ascontiguousarray(xg16[half:])

    in_maps = []
    for c in range(n_cores):
        in_maps.append(
            {
                "xlo": xlo,
                "xhi": xhi,
                "idx": idx16[c],
                "dstoff": dstoff[c],
                "iota": iota,
                "w4": w4,
                "b4": b4,
                "deg4": deg4[c],
                "nidx": nidx[c],
            }
        )
    meta = dict(nk2=nk2, seg_t0=seg_t0, chunks=chunks, T=T, ng=ng,
                nb=nb, cap=cap, half=half)
    return in_maps, meta


def _build(n_nodes, d, npc, meta):
    nk2, seg_t0, chunks, T, ng, nb, cap, half = (
        meta["nk2"], meta["seg_t0"], meta["chunks"], meta["T"],
        meta["ng"], meta["nb"], meta["cap"], meta["half"],
    )
    nc = bacc.Bacc("TRN2", num_swdge_queues=NQ)
    xlo_d = nc.dram_tensor("xlo", [half, d], GDT, kind="ExternalInput")
    xhi_d = nc.dram_tensor("xhi", [n_nodes - half, d], GDT, kind="ExternalInput")
    idx_d = nc.dram_tensor("idx", [P, 8 * T], I16, kind="ExternalInput")
    dstoff_d = nc.dram_tensor("dstoff", [P, T], ODT, kind="ExternalInput")
    iota_d = nc.dram_tensor("iota", [P, P], ODT, kind="ExternalInput")
    w4_d = nc.dram_tensor("w4", [d, d], F16, kind="ExternalInput")
    b4_d = nc.dram_tensor("b4", [1, d], F16, kind="ExternalInput")
    deg4_d = nc.dram_tensor("deg4", [1, ng * P], F16, kind="ExternalInput")
    nidx_d = nc.dram_tensor("nidx", [1, len(chunks)], I32,
                            kind="ExternalInput")
    out_d = nc.dram_tensor("out", [npc, d], F32, kind="ExternalOutput")

    x_in = [xlo_d, xhi_d]
    NCH = len(chunks)
    chunk_of = {}
    ci_of = {}
    for ci, (b, hh, t0c, nt, qn, groups) in enumerate(chunks):
        chunk_of[(b, hh)] = (t0c, nt, qn, groups)
        ci_of[(b, hh)] = ci

    with tile.TileContext(nc) as tc:
        with (
            tc.tile_pool(name="consts", bufs=1) as cpool,
            tc.tile_pool(name="xg", bufs=GBUFS) as gpool,
            tc.tile_pool(name="ind", bufs=6) as ipool,
            tc.tile_pool(name="ysb", bufs=3) as ypool,
            tc.tile_pool(name="osb", bufs=3) as opool,
            tc.tile_pool(name="ypsum", bufs=6, space="PSUM") as yppool,
            tc.tile_pool(name="opsum", bufs=2, space="PSUM") as oppool,
        ):
            # piecewise index tables as INDEPENDENT tiles (Tile tracks deps
            # per tile, so the first gather must not wait on the whole
            # table).  Piece boundaries align with chunk boundaries.
            npiece = 10
            target = -(-T // npiece)
            bndl = [0]
            for (_b, _hh, t0c, nt, _qn, _gr) in chunks:
                if t0c + nt - bndl[-1] >= target and t0c + nt < T:
                    bndl.append(t0c + nt)
            bndl.append(T)
            idx_p, dst_p = [], []
            for i in range(len(bndl) - 1):
                lo, hi = bndl[i], bndl[i + 1]
                ip = cpool.tile([P, 8 * (hi - lo)], I16, name=f"idxp{i}")
                nc.sync.dma_start(out=ip[:], in_=idx_d[:, 8 * lo : 8 * hi])
                dp = cpool.tile([P, hi - lo], ODT, name=f"dstp{i}")
                nc.sync.dma_start(out=dp[:], in_=dstoff_d[:, lo:hi])
                idx_p.append(ip)
                dst_p.append(dp)

            def piece_of(t):
                for i in range(len(bndl) - 1):
                    if bndl[i] <= t < bndl[i + 1]:
                        return i, bndl[i]
                raise AssertionError(t)

            iota_sb = cpool.tile([P, P], ODT)
            nc.sync.dma_start(out=iota_sb[:], in_=iota_d[:])
            w4_sb = cpool.tile([d, d], F16)
            nc.sync.dma_start(out=w4_sb[:], in_=w4_d[:])
            b4_sb = cpool.tile([1, d], F16)
            nc.sync.dma_start(out=b4_sb[:], in_=b4_d[:])
            deg4_sb = cpool.tile([1, ng * P], F16)
            nc.sync.dma_start(out=deg4_sb[:], in_=deg4_d[:])
            nidx_sb = cpool.tile([1, NCH], I32)
            nc.sync.dma_start(out=nidx_sb[:], in_=nidx_d[:])

            # num_idxs registers: double-buffered blocks so the block
            # reg_load only WARs against gathers two blocks back.
            NREGS = 12
            nidx_sets = [
                [nc.gpsimd.alloc_register(f"nidx_reg{s}_{i}")
                 for i in range(NREGS)]
                for s in (0, 1)
            ]

            # memset the gather buffers once so trailing pad slots (never
            # fetched) hold finite data; 0.0 * onehot(-1)==0 keeps them
            # inert.
            for i in range(GBUFS):
                zt = gpool.tile([P, cap * P], GDT, tag="xg", name=f"xgz{i}")
                nc.vector.memset(zt[:], 0.0)

            for b in range(nb):
                groups = list(range(b * BUNDLE, min(ng, (b + 1) * BUNDLE)))
                n_sub = {g: int(nk2[g][0] + nk2[g][1]) for g in groups}
                ypsums = {}
                for g in groups:
                    if n_sub[g] > 0:
                        ypsums[g] = yppool.tile([P, P], F32, tag="ypsum",
                                                name=f"ypsum_{g}")
                xgc = {}
                inds = {}
                ct0 = {}
                for hh in (0, 1):
                    if (b, hh) not in chunk_of:
                        continue
                    t0c, nt, qn, _gr = chunk_of[(b, hh)]
                    ci = ci_of[(b, hh)]
                    ct0[hh] = t0c
                    if ci % NREGS == 0:
                        n = min(NREGS, NCH - ci)
                        nc.gpsimd.reg_load(
                            nidx_sets[(ci // NREGS) % 2][:n],
                            nidx_sb[0:1, ci : ci + n],
                        )
                    nidx_reg = nidx_sets[(ci // NREGS) % 2][ci % NREGS]
                    pi, pb = piece_of(t0c)
                    assert t0c + nt <= bndl[pi + 1], "chunk straddles piece"
                    xgc[hh] = gpool.tile([P, cap * P], GDT, tag="xg",
                                         name=f"xg_{b}_{hh}")
                    nc.gpsimd.dma_gather(
                        out_ap=xgc[hh][:, : nt * P].rearrange(
                            "p (k f) -> p k f", f=P
                        ),
                        in_ap=x_in[hh][:],
                        idxs_ap=idx_p[pi][
                            :, 8 * (t0c - pb) : 8 * (t0c - pb + nt)
                        ],
                        num_idxs=nt * P,
                        num_idxs_reg=nidx_reg,
                        elem_size=P,
                        queue_num=qn,
                        single_packet=False,
                    )
                    # all of this chunk's subtile one-hots in ONE is_equal
                    ind = ipool.tile([P, cap * P], OHDT, tag="ind",
                                     name=f"ind_{b}_{hh}")
                    nc.vector.tensor_tensor(
                        out=ind[:, : nt * P].rearrange(
                            "p (c f) -> p c f", f=P
                        ),
                        in0=dst_p[pi][:, t0c - pb : t0c - pb + nt]
                        .rearrange("p c -> p c ()")
                        .to_broadcast([P, nt, P]),
                        in1=iota_sb[:]
                        .rearrange("p f -> p () f")
                        .to_broadcast([P, nt, P]),
                        op=mybir.AluOpType.is_equal,
                    )
                    inds[hh] = ind

                done = {g: 0 for g in groups}
                for hh in (0, 1):
                    if (b, hh) not in chunk_of:
                        continue
                    t0c = ct0[hh]
                    for g in groups:
                        for k in range(int(nk2[g][hh])):
                            koff = int(seg_t0[g][hh]) + k - t0c
                            nc.tensor.matmul(
                                out=ypsums[g][:],
                                lhsT=xgc[hh][:, koff * P : (koff + 1) * P],
                                rhs=inds[hh][:, koff * P : (koff + 1) * P],
                                start=(done[g] == 0),
                                stop=(done[g] == n_sub[g] - 1),
                            )
                            done[g] += 1

                for g in groups:
                    if n_sub[g] == 0:
                        continue
                    nv = min(P, npc - g * P)
                    ysb = ypool.tile([P, P], F16, tag="ysb", name=f"ysb_{g}")
                    nc.scalar.copy(out=ysb[:], in_=ypsums[g][:])
                    opsum = oppool.tile([P, P], F32, tag="opsum",
                                        name=f"opsum_{g}")
                    nc.tensor.matmul(
                        out=opsum[:], lhsT=ysb[:], rhs=w4_sb[:],
                        start=True, stop=False,
                    )
                    nc.tensor.matmul(
                        out=opsum[:],
                        lhsT=deg4_sb[:, g * P : (g + 1) * P],
                        rhs=b4_sb[:],
                        start=False,
                        stop=True,
                    )
                    osb = opool.tile([P, P], F32, tag="osb", name=f"osb_{g}")
                    nc.scalar.copy(out=osb[:], in_=opsum[:])
                    nc.sync.dma_start(
                        out=out_d[g * P : g * P + nv, :], in_=osb[:nv, :]
                    )

    nc.compile()
    return nc


def _run(inputs, trace=False, trace_kwargs=None):
    x = np.asarray(inputs["x"], np.float32)
    weight = np.asarray(inputs["weight"], np.float32)
    bias = np.asarray(inputs["bias"], np.float32)
    src = np.asarray(inputs["src"])
    dst = np.asarray(inputs["dst"])

    in_maps, meta = _prep(x, weight, bias, src, dst, N_CORES, NPC)
    nc = _build(N_NODES, D, NPC, meta)
    res = run_bass_kernel_spmd(
        nc,
        in_maps,
        list(range(N_CORES)),
        trace=trace,
        **(trace_kwargs or {}),
    )
    out = np.concatenate([res.results[c]["out"] for c in range(N_CORES)], axis=0)
    return out.reshape(N_NODES, HEADS, D // HEADS), res


def kernel(**inputs) -> np.ndarray:
    # the device occasionally comes up wedged from a prior run
    # (NRT_EXEC_UNIT_UNRECOVERABLE); a retry recovers it
    last = None
    for _ in range(3):
        try:
            out, _ = _run(inputs)
            return out
        except Exception as e:  # noqa: BLE001
            last = e
    raise last


# revision 22
# speedup vs baseline: 1.1996x; 1.0231x over previous
"""GAT layer kernel for Trainium2 (8 NeuronCores, SPMD).

Math note: in the reference,
    att = softmax(scores, axis=1); w = att.sum(axis=1)
sums the softmax over the exact axis it normalizes, so w == 1 identically
(up to fp rounding).  The layer therefore reduces to
    out[v] = (1/H) * ( (sum_{e: dst[e]==v} x[src[e]]) @ W  +  deg_in(v) * b )
i.e. a sparse neighbor aggregation (gather + segment-sum) followed by a
small dense matmul.  This is memory-bound in the per-edge row gather.

Sharding: edges are partitioned by dst-node range (6250 nodes per core), so
each core owns the full accumulation for its node slice and the final
output is a pure concatenation -- no inter-core collective needed.

Device pipeline per core (bottleneck structure, measured):
  - the SWDGE gather's idx ingestion is serialized on the Pool engine's
    shared read stream (~1.1 ns/idx), so the gather phase floor is set by
    total ingested idx slots; the DMA engines add ~20 ns per 256B row
    (16 engines).  Each dma_gather also burns a DMA-completion semaphore
    from a small rotating pool whose in-order reset waits on full DMA
    completion of the reusing gather's predecessor -- fewer, bigger
    gathers pipeline much better than many small ones.
  - edges are grouped by (2-group bundle, src half): 50 chunks, one
    dma_gather each, spread over 4 SWDGE queues (queue q = Q7 DSP pair
    2q,2q+1) with greedy load balancing.  Within a chunk the two groups'
    segments are 128-aligned; interior pad slots gather row 0, trailing
    pad slots carry idx -1 which the ucode trims (not fetched).  The
    per-core trimmed descriptor count is passed via num_idxs_reg
    (register, loaded in blocks of 12) so the decode-side ring
    bookkeeping matches the Q7-side generation exactly.
  - the x table is fp16: halves HBM bytes; accumulation stays fp32.
  - segment-sum on the tensor engine, one matmul per 128-edge subtile:
      ypsum_g[f, v] += xg[e, f]^T @ onehot[e, v]
    with ALL of a chunk's subtile one-hots built by a single wide
    vector-engine is_equal (dstoff column-block broadcast against an
    iota row), amortizing DVE instruction overhead ~10x.
  - per group, PSUM y is copied to fp16 and multiplied by W/4 (fp16,
    K=128 matmul) plus a K=1 fp16 outer-product matmul for the deg*b/4
    term (fp32 K=1 matmuls cost ~1.6us of PE each; fp16 is exact enough:
    deg/4 is a small exact quarter-integer), and streamed out.
"""

import numpy as np

import concourse.bass as bass
import concourse.tile as tile
from concourse import bacc, mybir
from concourse.bass_utils import run_bass_kernel_spmd

F32 = mybir.dt.float32
F16 = mybir.dt.float16
I16 = mybir.dt.int16
I32 = mybir.dt.int32
OHDT = mybir.dt.float16    # one-hot dtype
GDT = mybir.dt.float16     # gather-table dtype
GNP = np.float16
ODNP = np.float16          # dstoff/iota host dtype
ODT = mybir.dt.float16     # dstoff/iota device dtype

N_NODES = 50000
N_EDGES = 800000
D = 128          # in feats == H*F
HEADS = 4
N_CORES = 8
NPC = N_NODES // N_CORES      # nodes per core
P = 128                       # partitions / node-group size
BUNDLE = 2                    # groups per gather chunk
GBUFS = 12                    # gather buffers in flight
NQ = 4                        # SWDGE queues (DSP pairs)
NREGS = 12                    # num_idxs register block size


def _prep(x, weight, bias, src, dst, n_cores, npc):
    """Host-side sharding; returns per-core input maps + static tiling."""
    n_nodes, d = x.shape
    ng = (npc + P - 1) // P
    nb = (ng + BUNDLE - 1) // BUNDLE
    half = (n_nodes + 1) // 2
    assert half < 32768 and (n_nodes - half) < 32768

    src64 = src.astype(np.int64)
    dst64 = dst.astype(np.int64)
    core = dst64 // npc
    loc_node = dst64 % npc
    g_e = loc_node // P
    loc = (loc_node % P).astype(ODNP)
    h_e = (src64 >= half).astype(np.int64)

    key = (core * ng + g_e) * 2 + h_e
    order = np.argsort(key, kind="stable")
    key_s = key[order]
    src_s = src64[order]
    loc_s = loc[order]

    cnt = np.bincount(key, minlength=n_cores * ng * 2).reshape(n_cores, ng, 2)
    nk2 = (-(-cnt.max(axis=0) // P)).astype(np.int64)  # [ng, 2] tiles/segment

    # chunk list: (bundle, half) in issue order with greedy queue balance.
    # Within a chunk the member groups' segments are laid out consecutively
    # (128-aligned each).
    seg_t0 = np.zeros((ng, 2), np.int64)
    chunks = []  # (b, hh, t0, ntiles, queue, groups)
    qload = [0.0] * NQ
    qweight = [1.0, 1.0, 1.0, 1.0]
    t = 0
    for b in range(nb):
        groups = list(range(b * BUNDLE, min(ng, (b + 1) * BUNDLE)))
        for hh in (0, 1):
            t0c = t
            for g in groups:
                seg_t0[g, hh] = t
                t += int(nk2[g, hh])
            nt = t - t0c
            if nt == 0:
                continue
            qn = min(range(NQ), key=lambda q: qload[q] + nt * qweight[q])
            qload[qn] += nt * qweight[qn]
            chunks.append((b, hh, t0c, nt, qn, groups))
    T = t
    cap = max(nt for (_b, _hh, _t0, nt, _qn, _gr) in chunks)

    seg_start = np.zeros(n_cores * ng * 2, np.int64)
    seg_start[1:] = np.cumsum(cnt.reshape(-1))[:-1]
    rank = np.arange(len(src_s), dtype=np.int64) - seg_start[key_s]
    c_e = key_s // (ng * 2)
    gg_e = (key_s // 2) % ng
    t_e = seg_t0[gg_e, key_s % 2] + rank // P
    p_e = rank % P

    dstoff = np.full((n_cores, P, T), -1, ODNP)
    dstoff[c_e, p_e, t_e] = loc_s

    # int16 indices: slot (p, t) -> idx16[p % 16, 8*t + p//16].
    # Interior pads (non-final segment of a chunk) fetch row 0 (idx 0,
    # dstoff -1 keeps them inert); trailing pads of the final segment are
    # -1 (trimmed by the ucode -> not fetched).  The per-core trimmed
    # count goes to nidx for num_idxs_reg.
    idx16 = np.zeros((n_cores, 16, 8 * T), np.int16)
    hval = (src_s - (key_s % 2) * half).astype(np.int16)

    nidx = np.zeros((n_cores, 1, len(chunks)), np.int32)
    for ci, (b, hh, t0c, nt, _qn, groups) in enumerate(chunks):
        gl = groups[-1]
        last_t0 = int(seg_t0[gl, hh])          # global tile of last seg
        seg_tiles = int(nk2[gl, hh])
        for c in range(n_cores):
            cl = int(cnt[c, gl, hh])
            nidx[c, 0, ci] = (last_t0 - t0c) * P + cl
            for pos in range(cl, seg_tiles * P):
                tt, pp = last_t0 + pos // P, pos % P
                idx16[c, pp % 16, 8 * tt + pp // 16] = -1

    idx16[c_e, p_e % 16, 8 * t_e + p_e // 16] = hval
    idx16 = np.ascontiguousarray(np.tile(idx16, (1, 8, 1)))

    deg = np.bincount(dst64, minlength=n_nodes).astype(np.float32)
    deg4 = np.zeros((n_cores, 1, ng * P), GNP)
    deg4[:, 0, :npc] = (deg / HEADS).reshape(n_cores, npc).astype(GNP)

    iota = np.broadcast_to(np.arange(P, dtype=ODNP)[None, :], (P, P)).copy()
    w4 = np.ascontiguousarray((weight.astype(np.float32) / HEADS).astype(GNP))
    b4 = bias.astype(np.float32).reshape(1, d).astype(GNP)
    xg16 = np.ascontiguousarray(x.astype(GNP))
    xlo = np.ascontiguousarray(xg16[:half])
    xhi = np.ascontiguousarray(xg16[half:])

    in_maps = []
    for c in range(n_cores):
        in_maps.append(
            {
                "xlo": xlo,
                "xhi": xhi,
                "idx": idx16[c],
                "dstoff": dstoff[c],
                "iota": iota,
                "w4": w4,
                "b4": b4,
                "deg4": deg4[c],
                "nidx": nidx[c],
            }
        )
    meta = dict(nk2=nk2, seg_t0=seg_t0, chunks=chunks, T=T, ng=ng,
                nb=nb, cap=cap, half=half)
    return in_maps, meta


def _build(n_nodes, d, npc, meta):
    nk2, seg_t0, chunks, T, ng, nb, cap, half = (
        meta["nk2"], meta["seg_t0"], meta["chunks"], meta["T"],
        meta["ng"], meta["nb"], meta["cap"], meta["half"],
    )
    nc = bacc.Bacc("TRN2", num_swdge_queues=NQ)
    xlo_d = nc.dram_tensor("xlo", [half, d], GDT, kind="ExternalInput")
    xhi_d = nc.dram_tensor("xhi", [n_nodes - half, d], GDT, kind="ExternalInput")
    idx_d = nc.dram_tensor("idx", [P, 8 * T], I16, kind="ExternalInput")
    dstoff_d = nc.dram_tensor("dstoff", [P, T], ODT, kind="ExternalInput")
    iota_d = nc.dram_tensor("iota", [P, P], ODT, kind="ExternalInput")
    w4_d = nc.dram_tensor("w4", [d, d], F16, kind="ExternalInput")
    b4_d = nc.dram_tensor("b4", [1, d], F16, kind="ExternalInput")
    deg4_d = nc.dram_tensor("deg4", [1, ng * P], F16, kind="ExternalInput")
    nidx_d = nc.dram_tensor("nidx", [1, len(chunks)], I32,
                            kind="ExternalInput")
    out_d = nc.dram_tensor("out", [npc, d], F32, kind="ExternalOutput")

    x_in = [xlo_d, xhi_d]
    NCH = len(chunks)
    chunk_of = {}
    ci_of = {}
    for ci, (b, hh, t0c, nt, qn, groups) in enumerate(chunks):
        chunk_of[(b, hh)] = (t0c, nt, qn, groups)
        ci_of[(b, hh)] = ci

    with tile.TileContext(nc) as tc:
        with (
            tc.tile_pool(name="consts", bufs=1) as cpool,
            tc.tile_pool(name="xg", bufs=GBUFS) as gpool,
            tc.tile_pool(name="ind", bufs=6) as ipool,
            tc.tile_pool(name="ysb", bufs=3) as ypool,
            tc.tile_pool(name="osb", bufs=3) as opool,
            tc.tile_pool(name="ypsum", bufs=6, space="PSUM") as yppool,
            tc.tile_pool(name="opsum", bufs=2, space="PSUM") as oppool,
        ):
            # piecewise index tables as INDEPENDENT tiles (Tile tracks deps
            # per tile, so the first gather must not wait on the whole
            # table).  Piece boundaries align with chunk boundaries.
            npiece = 10
            target = -(-T // npiece)
            bndl = [0]
            for (_b, _hh, t0c, nt, _qn, _gr) in chunks:
                if t0c + nt - bndl[-1] >= target and t0c + nt < T:
                    bndl.append(t0c + nt)
            bndl.append(T)
            nidx_sb = cpool.tile([1, NCH], I32)
            nc.sync.dma_start(out=nidx_sb[:], in_=nidx_d[:])
            idx_p, dst_p = [], []
            for i in range(len(bndl) - 1):
                lo, hi = bndl[i], bndl[i + 1]
                ip = cpool.tile([P, 8 * (hi - lo)], I16, name=f"idxp{i}")
                nc.sync.dma_start(out=ip[:], in_=idx_d[:, 8 * lo : 8 * hi])
                dp = cpool.tile([P, hi - lo], ODT, name=f"dstp{i}")
                nc.sync.dma_start(out=dp[:], in_=dstoff_d[:, lo:hi])
                idx_p.append(ip)
                dst_p.append(dp)

            def piece_of(t):
                for i in range(len(bndl) - 1):
                    if bndl[i] <= t < bndl[i + 1]:
                        return i, bndl[i]
                raise AssertionError(t)

            iota_sb = cpool.tile([P, P], ODT)
            nc.sync.dma_start(out=iota_sb[:], in_=iota_d[:])
            w4_sb = cpool.tile([d, d], F16)
            nc.sync.dma_start(out=w4_sb[:], in_=w4_d[:])
            b4_sb = cpool.tile([1, d], F16)
            nc.sync.dma_start(out=b4_sb[:], in_=b4_d[:])
            deg4_sb = cpool.tile([1, ng * P], F16)
            nc.sync.dma_start(out=deg4_sb[:], in_=deg4_d[:])
            # num_idxs registers: double-buffered blocks so the block
            # reg_load only WARs against gathers two blocks back.
            NREGS = 12
            nidx_sets = [
                [nc.gpsimd.alloc_register(f"nidx_reg{s}_{i}")
                 for i in range(NREGS)]
                for s in (0, 1)
            ]

            # memset the gather buffers once so trailing pad slots (never
            # fetched) hold finite data; 0.0 * onehot(-1)==0 keeps them
            # inert.
            for i in range(GBUFS):
                zt = gpool.tile([P, cap * P], GDT, tag="xg", name=f"xgz{i}")
                if i % 2 == 0:
                    nc.vector.memset(zt[:], 0.0)
                else:
                    nc.scalar.memzero(zt[:])

            pending_out = []
            for b in range(nb):
                groups = list(range(b * BUNDLE, min(ng, (b + 1) * BUNDLE)))
                n_sub = {g: int(nk2[g][0] + nk2[g][1]) for g in groups}
                ypsums = {}
                for g in groups:
                    if n_sub[g] > 0:
                        ypsums[g] = yppool.tile([P, P], F32, tag="ypsum",
                                                name=f"ypsum_{g}")
                xgc = {}
                inds = {}
                ct0 = {}
                for hh in (0, 1):
                    if (b, hh) not in chunk_of:
                        continue
                    t0c, nt, qn, _gr = chunk_of[(b, hh)]
                    ci = ci_of[(b, hh)]
                    ct0[hh] = t0c
                    if ci % NREGS == 0:
                        n = min(NREGS, NCH - ci)
                        nc.gpsimd.reg_load(
                            nidx_sets[(ci // NREGS) % 2][:n],
                            nidx_sb[0:1, ci : ci + n],
                        )
                    nidx_reg = nidx_sets[(ci // NREGS) % 2][ci % NREGS]
                    pi, pb = piece_of(t0c)
                    assert t0c + nt <= bndl[pi + 1], "chunk straddles piece"
                    xgc[hh] = gpool.tile([P, cap * P], GDT, tag="xg",
                                         name=f"xg_{b}_{hh}")
                    nc.gpsimd.dma_gather(
                        out_ap=xgc[hh][:, : nt * P].rearrange(
                            "p (k f) -> p k f", f=P
                        ),
                        in_ap=x_in[hh][:],
                        idxs_ap=idx_p[pi][
                            :, 8 * (t0c - pb) : 8 * (t0c - pb + nt)
                        ],
                        num_idxs=nt * P,
                        num_idxs_reg=nidx_reg,
                        elem_size=P,
                        queue_num=qn,
                        single_packet=False,
                    )
                    # all of this chunk's subtile one-hots in ONE is_equal
                    ind = ipool.tile([P, cap * P], OHDT, tag="ind",
                                     name=f"ind_{b}_{hh}")
                    nc.vector.tensor_tensor(
                        out=ind[:, : nt * P].rearrange(
                            "p (c f) -> p c f", f=P
                        ),
                        in0=dst_p[pi][:, t0c - pb : t0c - pb + nt]
                        .rearrange("p c -> p c ()")
                        .to_broadcast([P, nt, P]),
                        in1=iota_sb[:]
                        .rearrange("p f -> p () f")
                        .to_broadcast([P, nt, P]),
                        op=mybir.AluOpType.is_equal,
                    )
                    inds[hh] = ind

                done = {g: 0 for g in groups}
                for hh in (0, 1):
                    if (b, hh) not in chunk_of:
                        continue
                    t0c = ct0[hh]
                    for g in groups:
                        for k in range(int(nk2[g][hh])):
                            koff = int(seg_t0[g][hh]) + k - t0c
                            nc.tensor.matmul(
                                out=ypsums[g][:],
                                lhsT=xgc[hh][:, koff * P : (koff + 1) * P],
                                rhs=inds[hh][:, koff * P : (koff + 1) * P],
                                start=(done[g] == 0),
                                stop=(done[g] == n_sub[g] - 1),
                            )
                            done[g] += 1

                # defer this bundle's output stage until after the NEXT
                # bundle's segsum matmuls: the W-matmul waits on the scalar
                # ysb copy, which would otherwise put a ~1us bubble in the
                # tensor queue every bundle (and a ~25us tensor backlog at
                # the end of the run).
                def _emit_out(groups=groups, n_sub=n_sub, ypsums=ypsums):
                    for g in groups:
                        if n_sub[g] == 0:
                            continue
                        nv = min(P, npc - g * P)
                        ysb = ypool.tile([P, P], F16, tag="ysb",
                                         name=f"ysb_{g}")
                        nc.scalar.copy(out=ysb[:], in_=ypsums[g][:])
                        opsum = oppool.tile([P, P], F32, tag="opsum",
                                            name=f"opsum_{g}")
                        nc.tensor.matmul(
                            out=opsum[:], lhsT=ysb[:], rhs=w4_sb[:],
                            start=True, stop=False,
                        )
                        nc.tensor.matmul(
                            out=opsum[:],
                            lhsT=deg4_sb[:, g * P : (g + 1) * P],
                            rhs=b4_sb[:],
                            start=False,
                            stop=True,
                        )
                        osb = opool.tile([P, P], F32, tag="osb",
                                         name=f"osb_{g}")
                        nc.scalar.copy(out=osb[:], in_=opsum[:])
                        nc.sync.dma_start(
                            out=out_d[g * P : g * P + nv, :], in_=osb[:nv, :]
                        )

                pending_out.append(_emit_out)
                if len(pending_out) > 1:
                    pending_out.pop(0)()

            for fn in pending_out:
                fn()

    nc.compile()
    return nc


def _run(inputs, trace=False, trace_kwargs=None):
    x = np.asarray(inputs["x"], np.float32)
    weight = np.asarray(inputs["weight"], np.float32)
    bias = np.asarray(inputs["bias"], np.float32)
    src = np.asarray(inputs["src"])
    dst = np.asarray(inputs["dst"])

    in_maps, meta = _prep(x, weight, bias, src, dst, N_CORES, NPC)
    nc = _build(N_NODES, D, NPC, meta)
    res = run_bass_kernel_spmd(
        nc,
        in_maps,
        list(range(N_CORES)),
        trace=trace,
        **(trace_kwargs or {}),
    )
    out = np.concatenate([res.results[c]["out"] for c in range(N_CORES)], axis=0)
    return out.reshape(N_NODES, HEADS, D // HEADS), res


def kernel(**inputs) -> np.ndarray:
    # the device occasionally comes up wedged from a prior run
    # (NRT_EXEC_UNIT_UNRECOVERABLE); a retry recovers it
    last = None
    for _ in range(3):
        try:
            out, _ = _run(inputs)
            return out
        except Exception as e:  # noqa: BLE001
            last = e
    raise last
